# revision 40
# baseline (speedup 1.0000x reference)
"""Trainium2 Bass kernel for nn_FCGAT (fully-connected GAT variant).

Mathematical simplifications used (exact, not approximate):

1. The reference computes
   ``out = einsum('nkj,nkd->nkd', softmax(aa, axis=2), h)`` which is
   ``h[n,k,d] * sum_j softmax(aa)[n,k,j] == h[n,k,d]``.  The whole attention
   block (z tensor, aw1/ab1/aw2/ab2, softmax) is dead code in real
   arithmetic; only float rounding noise (~1e-14 rel) distinguishes it.
   The model reduces to, per step::

       h_s = lrelu(lrelu([towers | x_s] @ w1.T + b1) @ w2.T + b2)
       x_{s+1} = h_s + x_s

   followed by ``prod_k sigmoid(x_K @ ow[0] + ob[0])`` over the K nodes of
   each graph.

2. The residual is distributed through the (linear) first matmul of the
   next step: ``x_s = x_0 + sum_{t<s} h_t``, so

       mm1(s) = w1 @ [towers | x_0]  +  sum_{t<s} w1[:, DT:] @ h_t
       logits = ow @ x_0 + sum_t ow @ h_t

   both as PSUM accumulation groups.  x_s is never materialized and there
   are no elementwise adds at all.

Sharding: data-parallel over the batch dim N=128 -> 16 graphs (1024 rows)
per core across 8 NeuronCores; all weights replicated.

Written in raw Bass (explicit engine blocks + semaphores) rather than Tile:
this toolchain's walrus build allows only ONE sync-wait command per
instruction, and Tile's auto-generated synchronization (per-DMA-queue
semaphores, slot-release waits, the kernel-tail drain) routinely needs
several.  Raw Bass sidesteps all of it: standalone single-condition wait
instructions and happens-before transitivity applied by construction.
Each DMA gets its own semaphore waited at full value only (concurrent DMA
completions are unordered, so intermediate shared-sem values are racy).

Performance notes:
  * linear-layer matmuls run as float32r (1 PE cycle/row at >=256 moving
    rows vs fp32's 4); every producer of a matmul operand writes a
    float32r-rounded output, which the BIR verifier requires;
  * the leaky_relu and sigmoid ACT tables never share an act-func-set
    (~1.3us load each), so both are prewarmed by dummy ops during the DMA
    phase;
  * the x0-part matmul of each accumulation group issues before the h2
    wait (only the h2-term matmul waits), hiding it under the eviction;
  * the two logits groups use separate PSUM banks (chunk 1 reuses the
    weight-transpose bank, long dead by then) so they don't serialize
    against the first sigmoid;
  * per-graph product = log2(K) tree of strided DVE multiplies (DVE
    TensorReduce has no mult op).

On-chip layout is feature-major ([feature, row] on partitions) so the
linear layers contract over the partition dim on the tensor engine.  Rows
are transposed on device with PE transposes (fp32 has no DMA transpose).
w1/w2 are pre-transposed on host inside the packed-constants array; the
DVE gives them (and ow) the float32r-rounding pass the verifier requires.
All constants arrive in one packed DMA on the SP HWDGE queue; x/towers
rows arrive in 4 two-tile DMAs split across the SP and ACT HWDGE queues.
"""

from contextlib import ExitStack

import numpy as np

import concourse.bass as bass
import concourse.mybir as mybir
from concourse.bass_utils import run_bass_kernel_spmd

N_CORES = 8
N, K, DT, D2 = 128, 64, 64, 64
D1 = DT + D2                # 128: [towers | x] feature dim
G = N // N_CORES            # 16 graphs per core
R = G * K                   # 1024 rows per core
CHUNK = 512                 # fp32 matmul moving-operand max
NCHUNK = R // CHUNK
PTILE = 128                 # rows per transpose tile
NTILE = R // PTILE
TILES_PER_DMA = 2
NDATA_DMA = NTILE // TILES_PER_DMA

# packed constants layout (columns of a [128, CW] f32 array)
C_ID = 0                    # 0:128    identity
C_W1 = 128                  # 128:256  w1.T (pre-transposed [d, o])
C_W2 = 256                  # 256:384  [0 | w2.T] (cols DT:D1 = w2.T)
C_B1 = 384                  # b1 column
C_B2 = 385                  # b2 in rows 64:128
C_OW = 386                  # ow[0] in rows 64:128, ob[0] at row 0
CW = 387

_F32 = mybir.dt.float32

# Results of the last hardware run (for the local test harness; the grading
# path only uses the return value of kernel()).
LAST_RESULT = None

_PROGRAM_CACHE = {}


def _build_program(kk: int, act_fn=None, use_fp32r=True) -> bass.Bass:
    LRELU = act_fn or mybir.ActivationFunctionType.Lrelu
    SIGMOID = mybir.ActivationFunctionType.Sigmoid

    def _r(ap):
        # float32r: same 4-byte storage; PE runs 1 cycle/row at >=256
        # moving rows instead of fp32's 4 (reduced internal precision).
        return ap.bitcast(mybir.dt.float32r) if use_fp32r else ap

    nc = bass.Bass()
    const_d = nc.declare_dram_parameter("cpack", [128, CW], _F32, isOutput=False)
    xc0_d = nc.declare_dram_parameter("xc0", [R, D1], _F32, isOutput=False)
    out_d = nc.declare_dram_parameter("out", [1, G], _F32, isOutput=True)

    # ---- instruction numbering (semaphore values), computed up front ----
    # PE: T_0..T_{NTILE-1}, then per step s: NCHUNK mm1 groups of (1+s)
    # matmuls followed by NCHUNK mm2s, then NCHUNK logits groups.
    pe = NTILE
    pe_mm1 = {}
    pe_mm2 = {}
    pe_mm3 = {}
    for s in range(kk):
        for c in range(NCHUNK):
            pe += 1 + s
            pe_mm1[(s, c)] = pe
        for c in range(NCHUNK):
            pe += 1
            pe_mm2[(s, c)] = pe
    for c in range(NCHUNK):
        pe += 1 + kk
        pe_mm3[c] = pe

    # ACT: two table-prewarm dummies (sigmoid, lrelu), then per step s:
    # h1(s,0..), h2(s,0..); finally sig(0..)
    ACT0 = 2

    def act_h1(s, c):
        return ACT0 + 2 * s * NCHUNK + c + 1

    def act_h2(s, c):
        return ACT0 + (2 * s + 1) * NCHUNK + c + 1

    def act_sig(c):
        return ACT0 + 2 * kk * NCHUNK + c + 1

    # DVE: w1t(1), w2t(2), ow-round(3), xcT 512-wide copies, then a
    # log2(K) tree of strided multiplies for the per-graph product
    NXCOPY = R // 512
    dve_xct = {c: 4 + (c * CHUNK) // 512 for c in range(NCHUNK)}
    NPROD = K.bit_length() - 1
    dve_prod = 3 + NXCOPY + 2 * NPROD

    with ExitStack() as ctx:
        cs = ctx.enter_context(nc.sbuf_tensor([128, CW], _F32))
        w1t = ctx.enter_context(nc.sbuf_tensor([D1, D1], _F32))
        w2t = ctx.enter_context(nc.sbuf_tensor([D1, D1], _F32))
        owf = ctx.enter_context(nc.sbuf_tensor([D1, 1], _F32))
        ldx = ctx.enter_context(nc.sbuf_tensor([PTILE, R], _F32))
        xcT = ctx.enter_context(nc.sbuf_tensor([D1, R], _F32))
        h1s = ctx.enter_context(
            nc.sbuf_tensor([D1, kk * NCHUNK * CHUNK + 1], _F32))
        h2s = ctx.enter_context(
            nc.sbuf_tensor([D1, kk * NCHUNK * CHUNK + 1], _F32))
        sig = ctx.enter_context(nc.sbuf_tensor([1, R], _F32))
        ptree = ctx.enter_context(nc.sbuf_tensor([1, R], _F32))
        prod = ctx.enter_context(nc.sbuf_tensor([1, G], _F32))
        warm = ctx.enter_context(nc.sbuf_tensor([1, 2], _F32))
        pst = ctx.enter_context(nc.psum_tensor([D1, R], _F32))
        ps_w = ctx.enter_context(nc.psum_tensor([128, 512], _F32))
        # full-bank allocations (a half-bank tensor could share a bank
        # with its neighbour -> fatal same-bank PE-write/engine-read overlap);
        # chunks rotate over two banks by parity
        ps1 = [ctx.enter_context(nc.psum_tensor(f"ps1_{p}", [D1, 512], _F32))
               for p in range(2)]
        ps2 = [ctx.enter_context(nc.psum_tensor(f"ps2_{p}", [D1, 512], _F32))
               for p in range(2)]
        ps3_0 = ctx.enter_context(nc.psum_tensor([1, 512], _F32))
        sem_const = ctx.enter_context(nc.semaphore("sem_const"))
        sem_data = [ctx.enter_context(nc.semaphore(f"sem_d{j}"))
                    for j in range(NDATA_DMA)]
        sem_out = ctx.enter_context(nc.semaphore("sem_out"))
        pe_sem = ctx.enter_context(nc.semaphore("pe_sem"))
        act_sem = ctx.enter_context(nc.semaphore("act_sem"))
        dve_sem = ctx.enter_context(nc.semaphore("dve_sem"))
        block = ctx.enter_context(nc.Block())

        ident = cs[:, C_ID:C_ID + 128]
        b1 = cs[:, C_B1:C_B1 + 1]
        b2 = cs[DT:D1, C_B2:C_B2 + 1]
        obc = cs[0:1, C_OW:C_OW + 1]
        owc = owf[DT:D1, :]
        # logits psum by chunk parity; odd chunks reuse the (long dead)
        # ps_w bank
        ps3 = [ps3_0[0:1, 0:CHUNK], ps_w[0:1, 0:CHUNK]]

        def ps1_ap(c):
            return ps1[c % 2][:, 0:CHUNK]

        def ps2_ap(c):
            return ps2[c % 2][:, 0:CHUNK]

        def h1_ap(s, c):
            off = (s * NCHUNK + c) * CHUNK
            return h1s[:, off:off + CHUNK]

        def h2_ap(s, c):
            # rows DT:D1 so SBUF operands share base partitions with
            # w1t[DT:], ow, and the mm2 psum rows
            off = (s * NCHUNK + c) * CHUNK
            return h2s[DT:D1, off:off + CHUNK]

        def data_dma(eng, j):
            rows = slice(j * TILES_PER_DMA * PTILE,
                         (j + 1) * TILES_PER_DMA * PTILE)
            eng.dma_start(
                ldx[:, rows],
                xc0_d[rows, :].rearrange("(t p) d -> p t d", p=PTILE),
            ).then_inc(sem_data[j], 16)

        @block.sync
        def _(sync):
            # first half of the data rides the ACT engine's HWDGE queue (see
            # block.scalar) so the two queues stream in parallel
            sync.dma_start(cs[:, :], const_d[:, :]).then_inc(sem_const, 16)
            for j in range(NDATA_DMA // 2, NDATA_DMA):
                data_dma(sync, j)
            sync.wait_ge(dve_sem, dve_prod)
            sync.dma_start(out_d[:, :], prod[:, :]).then_inc(sem_out, 16)
            sync.wait_ge(sem_out, 16)

        @block.tensor
        def _(tensor):
            wm = {}

            def twait(sem, val):
                # monotone watermark: skip waits already implied by an
                # earlier wait on the same semaphore
                if wm.get(id(sem), 0) < val:
                    wm[id(sem)] = val
                    tensor.wait_ge(sem, val)

            # the identity lives in the const pack
            twait(sem_const, 16)
            for t in range(NTILE):
                if t % TILES_PER_DMA == 0:
                    twait(sem_data[t // TILES_PER_DMA], 16)
                tsl = slice(t * PTILE, (t + 1) * PTILE)
                nc.tensor.transpose(
                    pst[:, tsl], ldx[:, tsl], ident
                ).then_inc(pe_sem, 1)
            for s in range(kk):
                for c in range(NCHUNK):
                    sl = slice(c * CHUNK, (c + 1) * CHUNK)
                    if s == 0:
                        twait(dve_sem, dve_xct[c])
                    # psum-bank WAR: the previous user of this parity bank
                    # must have been evicted (h1 of chunk c-2, or of the
                    # previous step's chunk c+2)
                    if c >= 2:
                        twait(act_sem, act_h1(s, c - 2))
                    elif s >= 1:
                        twait(act_sem, act_h1(s - 1, c + NCHUNK - 2))
                    # x0-part issues before the h2 wait
                    nc.tensor.matmul(
                        ps1_ap(c), _r(w1t[:, :]), _r(xcT[:, sl]),
                        start=True, stop=(s == 0),
                    ).then_inc(pe_sem, 1)
                    for t in range(s):
                        if t == s - 1:
                            twait(act_sem, act_h2(s - 1, c))
                        nc.tensor.matmul(
                            ps1_ap(c), _r(w1t[DT:D1, :]), _r(h2_ap(t, c)),
                            start=False, stop=(t == s - 1),
                        ).then_inc(pe_sem, 1)
                for c in range(NCHUNK):
                    twait(act_sem, act_h1(s, c))
                    if c >= 2:
                        twait(act_sem, act_h2(s, c - 2))
                    elif s >= 1:
                        twait(act_sem, act_h2(s - 1, c + NCHUNK - 2))
                    nc.tensor.matmul(
                        ps2_ap(c), _r(w2t[:, :]), _r(h1_ap(s, c)),
                        start=True, stop=True,
                    ).then_inc(pe_sem, 1)
            for c in range(NCHUNK):
                sl = slice(c * CHUNK, (c + 1) * CHUNK)
                if kk == 0:
                    twait(dve_sem, dve_xct[c])
                if c >= 2:
                    twait(act_sem, act_sig(c - 2))
                nc.tensor.matmul(
                    ps3[c % 2], _r(owc), _r(xcT[DT:D1, sl]),
                    start=True, stop=(kk == 0),
                ).then_inc(pe_sem, 1)
                for s in range(kk):
                    if s == kk - 1:
                        twait(act_sem, act_h2(kk - 1, c))
                    nc.tensor.matmul(
                        ps3[c % 2], _r(owc), _r(h2_ap(s, c)),
                        start=False, stop=(s == kk - 1),
                    ).then_inc(pe_sem, 1)

        @block.scalar
        def _(scalar):
            # first half of the data DMAs ride this engine's HWDGE queue,
            # in parallel with the SP queue (no waits: issue immediately)
            for j in range(NDATA_DMA // 2):
                data_dma(scalar, j)
            # Prewarm both ACT tables (leaky_relu and sigmoid never share an
            # act-func-set; each load is ~1.3us) immediately at t=0: the
            # input is the framework's preamble-memset const-0.0 cell, so no
            # DMA wait is needed and both loads finish well before the first
            # real eviction.  (Bias reads from `cs` are ordered behind the
            # const DMA transitively through each eviction's PE wait.)
            zcell = nc.const_aps.aps[(mybir.dt.float32, 0.0)][0:1, 0:1]
            nc.scalar.activation(
                warm[0:1, 0:1], zcell, SIGMOID
            ).then_inc(act_sem, 1)
            nc.scalar.activation(
                warm[0:1, 1:2], zcell, LRELU, alpha=0.01
            ).then_inc(act_sem, 1)
            seen = 0
            for s in range(kk):
                for c in range(NCHUNK):
                    if pe_mm1[(s, c)] > seen:
                        seen = pe_mm1[(s, c)]
                        scalar.wait_ge(pe_sem, seen)
                    nc.scalar.activation(
                        _r(h1_ap(s, c)), ps1_ap(c), LRELU,
                        bias=b1, alpha=0.01,
                    ).then_inc(act_sem, 1)
                for c in range(NCHUNK):
                    if pe_mm2[(s, c)] > seen:
                        seen = pe_mm2[(s, c)]
                        scalar.wait_ge(pe_sem, seen)
                    nc.scalar.activation(
                        _r(h2_ap(s, c)), ps2[c % 2][DT:D1, 0:CHUNK], LRELU,
                        bias=b2, alpha=0.01,
                    ).then_inc(act_sem, 1)
            for c in range(NCHUNK):
                sl = slice(c * CHUNK, (c + 1) * CHUNK)
                if pe_mm3[c] > seen:
                    seen = pe_mm3[c]
                    scalar.wait_ge(pe_sem, seen)
                nc.scalar.activation(
                    sig[0:1, sl], ps3[c % 2], SIGMOID, bias=obc
                ).then_inc(act_sem, 1)

        @block.vector
        def _(vector):
            # float32r-rounding passes over the host-pretransposed
            # weights (the verifier requires matmul operands to come from a
            # rounding instruction, which a DMA is not)
            vector.wait_ge(sem_const, 16)
            nc.vector.tensor_copy(
                out=_r(w1t[:, :]), in_=cs[:, C_W1:C_W1 + 128]
            ).then_inc(dve_sem, 1)
            nc.vector.tensor_copy(
                out=_r(w2t[:, :]), in_=cs[:, C_W2:C_W2 + 128]
            ).then_inc(dve_sem, 1)
            nc.vector.tensor_copy(
                out=_r(owc), in_=cs[DT:D1, C_OW:C_OW + 1]
            ).then_inc(dve_sem, 1)
            for c in range(NXCOPY):
                sl = slice(c * 512, (c + 1) * 512)
                vector.wait_ge(pe_sem, (c + 1) * (512 // PTILE))
                nc.vector.tensor_copy(
                    out=_r(xcT[:, sl]), in_=pst[:, sl]
                ).then_inc(dve_sem, 1)
            # per-graph product as two independent half-trees: the first
            # half (graphs of sigmoid chunk 0) runs while chunk 1's logits
            # and sigmoid are still in flight
            GH = G // 2
            dve_val = 3 + NXCOPY
            for hidx in range(2):
                vector.wait_ge(act_sem, act_sig(hidx * (NCHUNK // 2)
                                                + NCHUNK // 2 - 1))

                def gview(tensor, off, length):
                    ap = tensor[0:1, off:off + GH * length]
                    return ap.rearrange("p (g j) -> p g j", g=GH)

                prev_t, prev_off, half = sig, hidx * (R // 2), K // 2
                dst_off = hidx * (R // 2)
                first = True
                while half >= 1:
                    if not first:
                        # DVE completion is not implied by issue order;
                        # chained levels need an explicit completion wait
                        vector.wait_ge(dve_sem, dve_val)
                    pv = gview(prev_t, prev_off, 2 * half)
                    if half == 1:
                        dst = prod[0:1, hidx * GH:(hidx + 1) * GH].rearrange(
                            "p (g j) -> p g j", g=GH)
                    else:
                        dst = gview(ptree, dst_off, half)
                    nc.vector.tensor_tensor(
                        dst, pv[:, :, 0:half], pv[:, :, half:2 * half],
                        mybir.AluOpType.mult,
                    ).then_inc(dve_sem, 1)
                    dve_val += 1
                    first = False
                    prev_t, prev_off = ptree, dst_off
                    dst_off += GH * half
                    half //= 2

    return nc


def _pack_consts(w1, b1, w2, b2, ow, ob):
    cp = np.zeros((128, CW), np.float32)
    cp[:, C_ID:C_ID + 128] = np.eye(128, dtype=np.float32)
    # weights pre-transposed on host (raw Bass needs no on-device
    # const-sem staging, and PE transposes here would delay the data tiles)
    cp[:, C_W1:C_W1 + 128] = w1.T
    cp[:, C_W2 + DT:C_W2 + D1] = w2.T
    cp[:, C_B1] = b1
    cp[DT:D1, C_B2] = b2
    cp[DT:D1, C_OW] = ow.reshape(D2)
    cp[0, C_OW] = ob.reshape(())
    return cp


def _make_in_maps(towers, x, w1, b1, w2, b2, ow, ob):
    towers = np.asarray(towers, np.float32)
    x = np.asarray(x, np.float32)
    cpack = _pack_consts(
        np.asarray(w1, np.float32), np.asarray(b1, np.float32),
        np.asarray(w2, np.float32), np.asarray(b2, np.float32),
        np.asarray(ow, np.float32), np.asarray(ob, np.float32),
    )
    xc0 = np.concatenate(
        [towers.reshape(N * K, DT), x.reshape(N * K, D2)], axis=1
    )
    in_maps = []
    for i in range(N_CORES):
        sl = slice(i * R, (i + 1) * R)
        in_maps.append({"cpack": cpack, "xc0": np.ascontiguousarray(xc0[sl])})
    return in_maps


def kernel(towers, x, w1, b1, w2, b2, aw1, ab1, aw2, ab2, ow, ob, k):
    global LAST_RESULT
    kk = int(k)

    if kk not in _PROGRAM_CACHE:
        _PROGRAM_CACHE[kk] = _build_program(kk)
    nc = _PROGRAM_CACHE[kk]

    in_maps = _make_in_maps(towers, x, w1, b1, w2, b2, ow, ob)
    res = run_bass_kernel_spmd(nc, in_maps, list(range(N_CORES)))
    LAST_RESULT = res
    out = np.concatenate(
        [np.asarray(res.results[i]["out"]).reshape(G) for i in range(N_CORES)]
    )
    return out.astype(np.float32)
